# revision 3
# baseline (speedup 1.0000x reference)
"""Trainium2 Bass kernel for nn_Decoder_3289944948995 (GNN message-passing decoder).

Reference computation (per edge e):
    z   = concat(z_drug[row[e]], z_reaction[col[e]])          # [2H] = [1024]
    h   = relu(W1 @ z + b1)                                   # [512]
    out = W2 @ h + b2                                         # scalar

Algebraic restructure: W1 @ concat(zd, zr) = W1d @ zd + W1r @ zr, so with
per-h scale s = 0.5*|w2| folded into the node tables (host also reorders the
h axis so all w2>=0 columns come first, P of them):
    A[d] = s * (W1d @ z_drug[d] + b1)      # [2000, 512]  node table
    B[r] = s * (W1r @ z_reaction[r])       # [10000, 512] node table
    u    = A[row[e]] + B[col[e]]           # = 0.5*|w2|*y,  y = pre-relu acts
    out[e] = sum_h w2[h]*relu(y[h]) + b2
           = sum_{h<P} u + |u|  -  sum_{h>=P} u + |u|  + b2
(since w2*relu(y) = 0.5*(|w2|y + |w2||y|)*sign(w2)).  This removes the relu
pass entirely: the epilogue is one DVE add + four sliced DVE reduces (with
hardware abs/negate flags), no activation-engine pass, no PSUM in phase 2.

Phase 2 uses NON-transposed dma_gather: each edge's 512-fp16 node row is one
contiguous 1KB descriptor (the old transposed gather wrote 2-byte elements
across partitions, ~100x slower).  Layout out of the gather:
edge j of a 2048-edge tile sits at partition j%128, free chunk j//128.

Device schedule (identical SPMD program on 8 cores; core i owns edges
[i*50000, (i+1)*50000)):
  Phase 1: A/B tables on the PE from host-transposed fp16 z, scale+bias on
           DVE during the PSUM->SBUF copy, fp16 row-major tables to DRAM.
  Phase 2: per 2048-edge tile: dma_gather A[row] (queue 0) and B[col]
           (queue 1) -> [128, 16, 512] fp16; DVE add -> u; 4 DVE reduces
           (pos/neg x plain/abs) -> [128,16,4]; DVE sum -> [128,16];
           ACT +b2 into the resident output tile.  One 200KB DMA at the end.
"""

import numpy as np

H = 512
N_DRUG, N_REACTION, N_EDGES = 2000, 10000, 400000
N_CORES = 8
E_CORE = N_EDGES // N_CORES          # 50000 edges per core
ET = 2048                            # edges per gather tile
NT = -(-E_CORE // ET)                # 25 tiles
E_PAD = NT * ET                      # 51200 (padded with index 0)
IDX_COLS = ET // 16                  # 128 idx columns per tile
ECH = ET // 128                      # 16 edge chunks per tile
A_ROWS, B_ROWS = 2048, 10240         # node tables padded to 128 multiple
ZBLK = 1024                          # precompute node-block
KC = H // 128                        # 4 contraction chunks of 128

_CACHE = {}


def _build_nc(p_pos):
    import concourse.bacc as bacc
    import concourse.mybir as mybir
    import concourse.tile as tile
    from concourse import library_config
    from concourse.bass import ts

    dt = mybir.dt
    nc = bacc.Bacc(None, target_bir_lowering=False, num_swdge_queues=1)

    zdT = nc.dram_tensor("zdT", [H, A_ROWS], dt.float16, kind="ExternalInput")
    zrT = nc.dram_tensor("zrT", [H, B_ROWS], dt.float16, kind="ExternalInput")
    w1dT = nc.dram_tensor("w1dT", [H, H], dt.float16, kind="ExternalInput")
    w1rT = nc.dram_tensor("w1rT", [H, H], dt.float16, kind="ExternalInput")
    w2h_rep = nc.dram_tensor("w2h_rep", [128, H], dt.float32, kind="ExternalInput")
    b1s_rep = nc.dram_tensor("b1s_rep", [128, H], dt.float32, kind="ExternalInput")
    b2v = nc.dram_tensor("b2v", [128, 1], dt.float32, kind="ExternalInput")
    rowidx = nc.dram_tensor(
        "rowidx", [128, NT * IDX_COLS], dt.int16, kind="ExternalInput"
    )
    colidx = nc.dram_tensor(
        "colidx", [128, NT * IDX_COLS], dt.int16, kind="ExternalInput"
    )
    out = nc.dram_tensor("out", [128, NT * ECH], dt.float32, kind="ExternalOutput")

    with tile.TileContext(nc) as tc:
        with (
            tc.tile_pool(name="const", bufs=1) as cpool,
            tc.tile_pool(name="z", bufs=2) as zpool,
            tc.tile_pool(name="o1", bufs=3) as opool,
            tc.tile_pool(name="g", bufs=2) as gpool,
            tc.tile_pool(name="u", bufs=2) as upool,
            tc.tile_pool(name="acc", bufs=3) as apool,
            tc.tile_pool(name="ps1", bufs=4, space="PSUM") as ps1,
            tc.tile_pool(name="dram", bufs=1, space="DRAM") as dpool,
        ):
            # dma_gather (DMAGatherAnt) lives in the 'mlp' GPSIMD library
            nc.gpsimd.load_library(library_config.mlp)

            # ---- constant / index preload ----
            w1d_sb = cpool.tile([128, KC, H], dt.float16)
            nc.sync.dma_start(
                out=w1d_sb[:], in_=w1dT[:, :].rearrange("(c p) o -> p c o", p=128)
            )
            w1r_sb = cpool.tile([128, KC, H], dt.float16)
            nc.sync.dma_start(
                out=w1r_sb[:], in_=w1rT[:, :].rearrange("(c p) o -> p c o", p=128)
            )
            w2h_sb = cpool.tile([128, H], dt.float32)
            nc.sync.dma_start(out=w2h_sb[:], in_=w2h_rep[:, :])
            b1s_sb = cpool.tile([128, H], dt.float32)
            nc.sync.dma_start(out=b1s_sb[:], in_=b1s_rep[:, :])
            b2_sb = cpool.tile([128, 1], dt.float32)
            nc.sync.dma_start(out=b2_sb[:], in_=b2v[:, :])
            row_sb = cpool.tile([128, NT * IDX_COLS], dt.int16)
            nc.sync.dma_start(out=row_sb[:], in_=rowidx[:, :])
            col_sb = cpool.tile([128, NT * IDX_COLS], dt.int16)
            nc.sync.dma_start(out=col_sb[:], in_=colidx[:, :])
            out_sb = cpool.tile([128, NT, ECH], dt.float32)

            A_t = dpool.tile([A_ROWS, H], dt.float16, tag="A")
            B_t = dpool.tile([B_ROWS, H], dt.float16, tag="B")

            # ---- phase 1: node tables A = s*(zd@W1d.T + b1), B = s*(zr@W1r.T)
            def precompute(zT_handle, w1_sb, table, n_rows, add_b1):
                z_ap = zT_handle[:, :].rearrange(
                    "(c p) (b n) -> b p c n", p=128, n=ZBLK
                )
                for b in range(n_rows // ZBLK):
                    zt = zpool.tile([128, KC, ZBLK], dt.float16, tag="zt")
                    nc.sync.dma_start(out=zt[:], in_=z_ap[b])
                    for nt_ in range(ZBLK // 128):
                        psum = ps1.tile([128, H], dt.float32, tag="ps1")
                        for c in range(KC):
                            nc.tensor.matmul(
                                out=psum[:],
                                lhsT=zt[:, c, ts(nt_, 128)],
                                rhs=w1_sb[:, c, :],
                                start=(c == 0),
                                stop=(c == KC - 1),
                            )
                        osb = opool.tile([128, H], dt.float16, tag="osb")
                        if add_b1:
                            t32 = opool.tile([128, H], dt.float32, tag="t32")
                            nc.vector.tensor_mul(out=t32[:], in0=psum[:], in1=w2h_sb[:])
                            nc.vector.tensor_add(out=osb[:], in0=t32[:], in1=b1s_sb[:])
                        else:
                            nc.vector.tensor_mul(out=osb[:], in0=psum[:], in1=w2h_sb[:])
                        r0 = b * ZBLK + nt_ * 128
                        nc.sync.dma_start(out=table[r0 : r0 + 128, :], in_=osb[:])

            precompute(zdT, w1d_sb, A_t, A_ROWS, add_b1=True)
            precompute(zrT, w1r_sb, B_t, B_ROWS, add_b1=False)

            # ---- phase 2: per-edge gather + add + signed abs-reduce ----
            for t in range(NT):
                ag = gpool.tile([128, ECH, H], dt.float16, tag="ag")
                nc.gpsimd.dma_gather(
                    out_ap=ag[:],
                    in_ap=A_t[:, :],
                    idxs_ap=row_sb[:, ts(t, IDX_COLS)],
                    num_idxs=ET,
                    num_idxs_reg=ET,
                    elem_size=H,
                    transpose=False,
                    single_packet=False,
                )
                bg = gpool.tile([128, ECH, H], dt.float16, tag="bg")
                nc.gpsimd.dma_gather(
                    out_ap=bg[:],
                    in_ap=B_t[:, :],
                    idxs_ap=col_sb[:, ts(t, IDX_COLS)],
                    num_idxs=ET,
                    num_idxs_reg=ET,
                    elem_size=H,
                    transpose=False,
                    single_packet=False,
                )
                u = upool.tile([128, ECH, H], dt.float16, tag="u")
                nc.vector.tensor_add(out=u[:], in0=ag[:], in1=bg[:])
                acc = apool.tile([128, ECH, 4], dt.float32, tag="acc")
                slices = []
                if p_pos > 0:
                    slices.append((slice(0, p_pos), False))
                if p_pos < H:
                    slices.append((slice(p_pos, H), True))
                k = 0
                for hsl, neg in slices:
                    for use_abs in (False, True):
                        nc.vector.tensor_reduce(
                            out=acc[:, :, k],
                            in_=u[:, :, hsl],
                            axis=mybir.AxisListType.X,
                            op=mybir.AluOpType.add,
                            apply_absolute_value=use_abs,
                            negate=neg,
                        )
                        k += 1
                for _ in range(k, 4):  # zero unused lanes (degenerate sign split)
                    nc.vector.memset(acc[:, :, k], 0.0)
                    k += 1
                s = apool.tile([128, ECH], dt.float32, tag="s")
                nc.vector.tensor_reduce(
                    out=s[:],
                    in_=acc[:],
                    axis=mybir.AxisListType.X,
                    op=mybir.AluOpType.add,
                )
                # + b2 (per-partition scalar bias) into the resident out tile
                nc.scalar.activation(
                    out=out_sb[:, t, :],
                    in_=s[:],
                    func=mybir.ActivationFunctionType.Identity,
                    bias=b2_sb[:, :],
                )
            nc.sync.dma_start(
                out=out[:, :], in_=out_sb[:].rearrange("p t c -> p (t c)")
            )
    nc.compile()
    return nc


def _wrap_idx(a):
    """[E_PAD] int -> [128, NT*IDX_COLS] int16 in dma_gather's wrapped layout.

    Within tile t, index j (0..ET-1) sits at partition j%16 (replicated to all
    8 groups of 16 partitions), free column t*IDX_COLS + j//16.
    """
    m = a.reshape(NT, IDX_COLS, 16)          # [t, j//16, j%16]
    w = m.transpose(0, 2, 1)                 # [t, 16, IDX_COLS]
    w = np.tile(w, (1, 8, 1))                # [t, 128, IDX_COLS]
    w = w.transpose(1, 0, 2).reshape(128, NT * IDX_COLS)
    return np.ascontiguousarray(w, dtype=np.int16)


def unwrap_out(a):
    """[128, NT*ECH] device layout -> [E_PAD] edge order."""
    return (
        np.asarray(a).reshape(128, NT, ECH).transpose(1, 2, 0).reshape(-1)
    )


def get_nc():
    assert "p_pos" in _CACHE, "call make_in_maps first"
    key = ("nc", _CACHE["p_pos"])
    if key not in _CACHE:
        _CACHE[key] = _build_nc(_CACHE["p_pos"])
    return _CACHE[key]


def make_in_maps(z_drug, z_reaction, row, col, W1, b1, W2, b2):
    f16 = np.float16
    w2 = np.asarray(W2, np.float64).reshape(-1)              # [512]
    perm = np.argsort(w2 < 0, kind="stable")                 # w2>=0 first
    p_pos = int((w2 >= 0).sum())
    _CACHE["p_pos"] = p_pos
    scale = (0.5 * np.abs(w2[perm])).astype(np.float32)      # [512]

    zdT = np.zeros((H, A_ROWS), f16)
    zdT[:, :N_DRUG] = np.asarray(z_drug, np.float32).T.astype(f16)
    zrT = np.zeros((H, B_ROWS), f16)
    zrT[:, :N_REACTION] = np.asarray(z_reaction, np.float32).T.astype(f16)
    W1p = np.asarray(W1, np.float32)[perm, :]                # reorder out-h
    w1dT = np.ascontiguousarray(W1p[:, :H].T).astype(f16)
    w1rT = np.ascontiguousarray(W1p[:, H:].T).astype(f16)
    w2h_rep = np.ascontiguousarray(np.broadcast_to(scale[None, :], (128, H)))
    b1s = (scale * np.asarray(b1, np.float32)[perm]).astype(np.float32)
    b1s_rep = np.ascontiguousarray(np.broadcast_to(b1s[None, :], (128, H)))
    b2v = np.full((128, 1), float(np.asarray(b2).reshape(-1)[0]), np.float32)
    row = np.asarray(row).astype(np.int64)
    col = np.asarray(col).astype(np.int64)

    in_maps = []
    for ci in range(N_CORES):
        sl = slice(ci * E_CORE, (ci + 1) * E_CORE)
        r = np.zeros(E_PAD, np.int64)
        r[:E_CORE] = row[sl]
        c = np.zeros(E_PAD, np.int64)
        c[:E_CORE] = col[sl]
        in_maps.append(
            {
                "zdT": zdT,
                "zrT": zrT,
                "w1dT": w1dT,
                "w1rT": w1rT,
                "w2h_rep": w2h_rep,
                "b1s_rep": b1s_rep,
                "b2v": b2v,
                "rowidx": _wrap_idx(r),
                "colidx": _wrap_idx(c),
            }
        )
    return in_maps


def kernel(z_drug, z_reaction, row, col, W1, b1, W2, b2):
    from concourse.bass_utils import run_bass_kernel_spmd

    in_maps = make_in_maps(z_drug, z_reaction, row, col, W1, b1, W2, b2)
    nc = get_nc()
    res = run_bass_kernel_spmd(nc, in_maps, core_ids=list(range(N_CORES)))
    outs = [unwrap_out(r["out"])[:E_CORE] for r in res.results]
    return np.ascontiguousarray(np.concatenate(outs), dtype=np.float32)


# revision 4
# speedup vs baseline: 1.0297x; 1.0297x over previous
"""Trainium2 Bass kernel for nn_Decoder_3289944948995 (GNN message-passing decoder).

Reference computation (per edge e):
    z   = concat(z_drug[row[e]], z_reaction[col[e]])          # [2H] = [1024]
    h   = relu(W1 @ z + b1)                                   # [512]
    out = W2 @ h + b2                                         # scalar

Algebraic restructure: W1 @ concat(zd, zr) = W1d @ zd + W1r @ zr, so with
per-h scale s = 0.5*|w2| folded into the node tables (host also reorders the
h axis so all w2>=0 columns come first, P of them):
    A[d] = s * (W1d @ z_drug[d] + b1)      # [2000, 512]  node table
    B[r] = s * (W1r @ z_reaction[r])       # [10000, 512] node table
    u    = A[row[e]] + B[col[e]]           # = 0.5*|w2|*y,  y = pre-relu acts
    out[e] = sum_{h<P} u + |u|  -  sum_{h>=P} u + |u|  + b2
(since w2*relu(y) = 0.5*(|w2|y + |w2||y|)*sign(w2)).  No relu pass: the
epilogue is one in-place DVE add + four sliced DVE reduces (hw abs/negate).

Gathers are NON-transposed dma_gather (one contiguous 1KB descriptor per
edge-row; the old transposed gather wrote 2-byte elements, ~100x slower).
A and B live in ONE DRAM table T=[A;B] (12288 rows) so each tile needs a
single gather of 2*ET indices: A-slot j=k, B-slot j=ET+k for edge k, which
lands pairs on the same partition (edge k -> partition k%128).

Overlap trick: edges are sorted by col (reaction id) and dealt round-robin
to the 8 cores, so every core's tile t touches only B rows < rdep[t], a
compile-time constant (same for all cores).  Each gather's in_ap is the
prefix T[0 : 2048+rdep[t]], so the Tile framework's range-based hazard
tracking lets early tiles' gathers run while the PE is still producing
later B blocks.  Host un-permutes the output.
"""

import numpy as np

H = 512
N_DRUG, N_REACTION, N_EDGES = 2000, 10000, 400000
N_CORES = 8
E_CORE = N_EDGES // N_CORES          # 50000 edges per core
ET = 3584                            # edges per gather tile
NT = -(-E_CORE // ET)                # 14 tiles
E_PAD = NT * ET                      # 50176 (padded with index 0)
NI = 2 * ET                          # 7168 gathered rows per tile
IDX_COLS = NI // 16                  # 448 idx columns per tile
ECH = ET // 128                      # 28 edge chunks per tile
A_ROWS, B_ROWS = 2048, 10240         # node tables padded to 128 multiple
T_ROWS = A_ROWS + B_ROWS
ZBLK = 1024                          # precompute node-block
KC = H // 128                        # 4 contraction chunks of 128

_CACHE = {}


def _build_nc(p_pos, rdep):
    import concourse.bacc as bacc
    import concourse.mybir as mybir
    import concourse.tile as tile
    from concourse import library_config
    from concourse.bass import ts

    dt = mybir.dt
    nc = bacc.Bacc(None, target_bir_lowering=False)

    zdT = nc.dram_tensor("zdT", [H, A_ROWS], dt.float16, kind="ExternalInput")
    zrT = nc.dram_tensor("zrT", [H, B_ROWS], dt.float16, kind="ExternalInput")
    w1dT = nc.dram_tensor("w1dT", [H, H], dt.float16, kind="ExternalInput")
    w1rT = nc.dram_tensor("w1rT", [H, H], dt.float16, kind="ExternalInput")
    w2h_rep = nc.dram_tensor("w2h_rep", [128, H], dt.float32, kind="ExternalInput")
    b1s_rep = nc.dram_tensor("b1s_rep", [128, H], dt.float32, kind="ExternalInput")
    b2v = nc.dram_tensor("b2v", [128, 1], dt.float32, kind="ExternalInput")
    jidx = nc.dram_tensor(
        "jidx", [128, NT * IDX_COLS], dt.int16, kind="ExternalInput"
    )
    out = nc.dram_tensor("out", [128, NT * ECH], dt.float32, kind="ExternalOutput")

    with tile.TileContext(nc) as tc:
        with (
            tc.tile_pool(name="const", bufs=1) as cpool,
            tc.tile_pool(name="z", bufs=2) as zpool,
            tc.tile_pool(name="o1", bufs=3) as opool,
            tc.tile_pool(name="g", bufs=2) as gpool,
            tc.tile_pool(name="acc", bufs=3) as apool,
            tc.tile_pool(name="ps1", bufs=4, space="PSUM") as ps1,
            tc.tile_pool(name="dram", bufs=1, space="DRAM") as dpool,
        ):
            # dma_gather (DMAGatherAnt) lives in the 'mlp' GPSIMD library
            nc.gpsimd.load_library(library_config.mlp)

            # ---- constant / index preload ----
            w1d_sb = cpool.tile([128, KC, H], dt.float16)
            nc.sync.dma_start(
                out=w1d_sb[:], in_=w1dT[:, :].rearrange("(c p) o -> p c o", p=128)
            )
            w1r_sb = cpool.tile([128, KC, H], dt.float16)
            nc.sync.dma_start(
                out=w1r_sb[:], in_=w1rT[:, :].rearrange("(c p) o -> p c o", p=128)
            )
            w2h_sb = cpool.tile([128, H], dt.float32)
            nc.sync.dma_start(out=w2h_sb[:], in_=w2h_rep[:, :])
            b1s_sb = cpool.tile([128, H], dt.float32)
            nc.sync.dma_start(out=b1s_sb[:], in_=b1s_rep[:, :])
            b2_sb = cpool.tile([128, 1], dt.float32)
            nc.sync.dma_start(out=b2_sb[:], in_=b2v[:, :])
            idx_sb = cpool.tile([128, NT * IDX_COLS], dt.int16)
            nc.sync.dma_start(out=idx_sb[:], in_=jidx[:, :])
            out_sb = cpool.tile([128, NT, ECH], dt.float32)

            T_t = dpool.tile([T_ROWS, H], dt.float16, tag="T")

            # ---- phase 1: T[0:2048]=s*(zd@W1d.T+b1); T[2048:]=s*(zr@W1r.T)
            def precompute(zT_handle, w1_sb, base, n_rows, add_b1):
                z_ap = zT_handle[:, :].rearrange(
                    "(c p) (b n) -> b p c n", p=128, n=ZBLK
                )
                for b in range(n_rows // ZBLK):
                    zt = zpool.tile([128, KC, ZBLK], dt.float16, tag="zt")
                    nc.sync.dma_start(out=zt[:], in_=z_ap[b])
                    for nt_ in range(ZBLK // 128):
                        psum = ps1.tile([128, H], dt.float32, tag="ps1")
                        for c in range(KC):
                            nc.tensor.matmul(
                                out=psum[:],
                                lhsT=zt[:, c, ts(nt_, 128)],
                                rhs=w1_sb[:, c, :],
                                start=(c == 0),
                                stop=(c == KC - 1),
                            )
                        osb = opool.tile([128, H], dt.float16, tag="osb")
                        if add_b1:
                            t32 = opool.tile([128, H], dt.float32, tag="t32")
                            nc.vector.tensor_mul(out=t32[:], in0=psum[:], in1=w2h_sb[:])
                            nc.vector.tensor_add(out=osb[:], in0=t32[:], in1=b1s_sb[:])
                        else:
                            nc.vector.tensor_mul(out=osb[:], in0=psum[:], in1=w2h_sb[:])
                        r0 = base + b * ZBLK + nt_ * 128
                        nc.sync.dma_start(out=T_t[r0 : r0 + 128, :], in_=osb[:])

            precompute(zdT, w1d_sb, 0, A_ROWS, add_b1=True)
            precompute(zrT, w1r_sb, A_ROWS, B_ROWS, add_b1=False)

            # ---- phase 2: per-tile combined gather + add + signed abs-reduce
            for t in range(NT):
                g = gpool.tile([128, 2 * ECH, H], dt.float16, tag="g")
                nc.gpsimd.dma_gather(
                    out_ap=g[:],
                    in_ap=T_t[0 : A_ROWS + rdep[t], :],
                    idxs_ap=idx_sb[:, ts(t, IDX_COLS)],
                    num_idxs=NI,
                    num_idxs_reg=NI,
                    elem_size=H,
                    transpose=False,
                    single_packet=False,
                )
                u = g[:, 0:ECH, :]
                nc.vector.tensor_add(out=u, in0=u, in1=g[:, ECH : 2 * ECH, :])
                acc = apool.tile([128, ECH, 4], dt.float32, tag="acc")
                slices = []
                if p_pos > 0:
                    slices.append((slice(0, p_pos), False))
                if p_pos < H:
                    slices.append((slice(p_pos, H), True))
                k = 0
                for hsl, neg in slices:
                    for use_abs in (False, True):
                        nc.vector.tensor_reduce(
                            out=acc[:, :, k],
                            in_=u[:, :, hsl],
                            axis=mybir.AxisListType.X,
                            op=mybir.AluOpType.add,
                            apply_absolute_value=use_abs,
                            negate=neg,
                        )
                        k += 1
                for _ in range(k, 4):  # zero unused lanes (degenerate sign split)
                    nc.vector.memset(acc[:, :, k], 0.0)
                    k += 1
                s = apool.tile([128, ECH], dt.float32, tag="s")
                nc.vector.tensor_reduce(
                    out=s[:],
                    in_=acc[:],
                    axis=mybir.AxisListType.X,
                    op=mybir.AluOpType.add,
                )
                # + b2 (per-partition scalar bias) into the resident out tile
                nc.scalar.activation(
                    out=out_sb[:, t, :],
                    in_=s[:],
                    func=mybir.ActivationFunctionType.Identity,
                    bias=b2_sb[:, :],
                )
            nc.sync.dma_start(
                out=out[:, :], in_=out_sb[:].rearrange("p t c -> p (t c)")
            )
    nc.compile()
    return nc


def _wrap_idx(a):
    """[NT*NI] int -> [128, NT*IDX_COLS] int16 in dma_gather's wrapped layout.

    Within tile t, index j (0..NI-1) sits at partition j%16 (replicated to all
    8 groups of 16 partitions), free column t*IDX_COLS + j//16.
    """
    m = a.reshape(NT, IDX_COLS, 16)          # [t, j//16, j%16]
    w = m.transpose(0, 2, 1)                 # [t, 16, IDX_COLS]
    w = np.tile(w, (1, 8, 1))                # [t, 128, IDX_COLS]
    w = w.transpose(1, 0, 2).reshape(128, NT * IDX_COLS)
    return np.ascontiguousarray(w, dtype=np.int16)


def unwrap_out(a):
    """[128, NT*ECH] device layout -> [E_PAD] per-core edge order."""
    return np.asarray(a).reshape(128, NT, ECH).transpose(1, 2, 0).reshape(-1)


def get_nc():
    assert "p_pos" in _CACHE, "call make_in_maps first"
    key = ("nc", _CACHE["p_pos"], _CACHE["rdep"])
    if key not in _CACHE:
        _CACHE[key] = _build_nc(_CACHE["p_pos"], _CACHE["rdep"])
    return _CACHE[key]


def core_edge_ids(ci):
    """Global edge indices handled by core ci, in its device order."""
    return _CACHE["order"][ci::N_CORES]


def make_in_maps(z_drug, z_reaction, row, col, W1, b1, W2, b2):
    f16 = np.float16
    w2 = np.asarray(W2, np.float64).reshape(-1)              # [512]
    perm = np.argsort(w2 < 0, kind="stable")                 # w2>=0 first
    p_pos = int((w2 >= 0).sum())
    _CACHE["p_pos"] = p_pos
    scale = (0.5 * np.abs(w2[perm])).astype(np.float32)      # [512]

    zdT = np.zeros((H, A_ROWS), f16)
    zdT[:, :N_DRUG] = np.asarray(z_drug, np.float32).T.astype(f16)
    zrT = np.zeros((H, B_ROWS), f16)
    zrT[:, :N_REACTION] = np.asarray(z_reaction, np.float32).T.astype(f16)
    W1p = np.asarray(W1, np.float32)[perm, :]                # reorder out-h
    w1dT = np.ascontiguousarray(W1p[:, :H].T).astype(f16)
    w1rT = np.ascontiguousarray(W1p[:, H:].T).astype(f16)
    w2h_rep = np.ascontiguousarray(np.broadcast_to(scale[None, :], (128, H)))
    b1s = (scale * np.asarray(b1, np.float32)[perm]).astype(np.float32)
    b1s_rep = np.ascontiguousarray(np.broadcast_to(b1s[None, :], (128, H)))
    b2v = np.full((128, 1), float(np.asarray(b2).reshape(-1)[0]), np.float32)
    row = np.asarray(row).astype(np.int64)
    col = np.asarray(col).astype(np.int64)

    # sort edges by col; deal round-robin to cores so every core's tile t
    # covers the same B-prefix rdep[t] (compile-time, SPMD-uniform)
    order = np.argsort(col, kind="stable")
    _CACHE["order"] = order
    col_sorted = col[order]
    rdep = []
    for t in range(NT):
        hi = min(N_CORES * ET * (t + 1), N_EDGES) - 1
        r = int(col_sorted[hi]) + 1
        rdep.append(min(-(-r // 128) * 128, B_ROWS))
    _CACHE["rdep"] = tuple(rdep)

    in_maps = []
    for ci in range(N_CORES):
        ids = order[ci::N_CORES]
        r = np.zeros(E_PAD, np.int64)
        r[:E_CORE] = row[ids]
        c = np.zeros(E_PAD, np.int64)
        c[:E_CORE] = col[ids]
        # combined-table slots: tile t, edge k -> A at j=k, B at j=ET+k
        j = np.zeros(NT * NI, np.int64)
        jt = j.reshape(NT, NI)
        jt[:, :ET] = r.reshape(NT, ET)
        jt[:, ET:] = A_ROWS + c.reshape(NT, ET)
        in_maps.append(
            {
                "zdT": zdT,
                "zrT": zrT,
                "w1dT": w1dT,
                "w1rT": w1rT,
                "w2h_rep": w2h_rep,
                "b1s_rep": b1s_rep,
                "b2v": b2v,
                "jidx": _wrap_idx(j),
            }
        )
    return in_maps


def kernel(z_drug, z_reaction, row, col, W1, b1, W2, b2):
    from concourse.bass_utils import run_bass_kernel_spmd

    in_maps = make_in_maps(z_drug, z_reaction, row, col, W1, b1, W2, b2)
    nc = get_nc()
    res = run_bass_kernel_spmd(nc, in_maps, core_ids=list(range(N_CORES)))
    out_full = np.empty(N_EDGES, np.float32)
    for ci, r in enumerate(res.results):
        out_full[core_edge_ids(ci)] = unwrap_out(r["out"])[:E_CORE]
    return np.ascontiguousarray(out_full)
